# revision 20
# baseline (speedup 1.0000x reference)
"""DualRelGCN message-passing kernel for 8 TRN2 NeuronCores.

Strategy (destination-sharded, collective-free, distinct-src folding):
  - LayerNorm is invariant to positive per-row scaling, so LN(agg/denom) ==
    LN(agg): the denominator drops out of the computation entirely.
  - Shard edges by dst range: core c owns nodes [1250c, 1250(c+1)) and
    computes its 1250 output rows completely locally -> no collectives.
  - Per dst tile (128 dsts) only ~3.1k of the 10k src nodes have an edge in,
    so the host compacts the active src rows into <=28 "fold" tiles of 128:
    X~[fold] = gathered rel_embed rows (fp8), W~[fold] = [128 src x 128 dst]
    dense weight block over the compacted rows.  agg_t = sum_f W~_f.T X~_f.
    vs. the all-src dense formulation this is ~3x fewer PE block-matmuls
    (the PE runs DoubleRow fp8 at ~256 cycles per 2-tile pair on this hw)
    and ~15% fewer HBM bytes.
  - X~ row i and its weight row ride adjacent in one interleaved stream
    ([128, fold, 256+128] fp8), DMA'd in exact PE consumption order, fully
    SBUF-resident -> single full-bandwidth burst, no recycle stalls.
  - Epilogue per dst tile: LN on ACT/DVE, PE transpose, y = ln @ proj_w.T,
    out = rel + 0.1*y in bf16 (host upcasts to fp32).
"""

import sys

for _p in ("/opt/trn_rl_repo",):
    if _p not in sys.path:
        sys.path.insert(0, _p)

from contextlib import ExitStack

import numpy as np
import ml_dtypes

import concourse.bacc as bacc
import concourse.mybir as mybir
from concourse.alu_op_type import AluOpType
from concourse.tile import TileContext
from concourse.bass_utils import run_bass_kernel_spmd

F32 = mybir.dt.float32
BF16 = mybir.dt.bfloat16
FP8 = mybir.dt.float8e4
AF = mybir.ActivationFunctionType

N_NODES = 10000
DIM = 256
N_CORES = 8
NODES_PER_CORE = N_NODES // N_CORES  # 1250
TILE = 128
N_TILES = -(-NODES_PER_CORE // TILE)  # 10 dst tiles per core
OUT_ROWS = N_TILES * TILE  # 1280
ALPHA = 0.1
LN_EPS = 1e-5

_CACHE: dict = {}


def _chunks(t, f):
    # fold-unit chunk boundaries within tile t: halves, finer at the ends
    if t == 0:
        return [0, 8, f]
    if t == N_TILES - 1:
        return [0, f // 2, (f // 2 + f) // 2, f]
    return [0, f // 2, f]


def _build(f_ts):
    off = [0]
    for f in f_ts:
        off.append(off[-1] + f)
    totf = off[-1]
    nc = bacc.Bacc("TRN2", target_bir_lowering=False, debug=False,
                   num_devices=N_CORES)

    xt_d = nc.dram_tensor("xt", [128, totf * DIM], FP8,
                          kind="ExternalInput")
    wt_d = nc.dram_tensor("wt", [128, totf * TILE], FP8,
                          kind="ExternalInput")
    ident_d = nc.dram_tensor("ident", [128, 128], BF16, kind="ExternalInput")
    relsl_d = nc.dram_tensor("relslice", [128, N_TILES * DIM], BF16,
                             kind="ExternalInput")
    pwt_d = nc.dram_tensor("projwT", [128, 2 * DIM], BF16,
                           kind="ExternalInput")
    out_d = nc.dram_tensor("out", [128, N_TILES * DIM], BF16,
                           kind="ExternalOutput")

    with TileContext(nc) as tc, ExitStack() as es:
        const_pool = es.enter_context(tc.tile_pool(name="const", bufs=1))
        ep_pool = es.enter_context(tc.tile_pool(name="ep", bufs=2))
        ps_agg = es.enter_context(tc.tile_pool(name="ps_agg", bufs=4,
                                               space="PSUM"))
        ps_tr = es.enter_context(tc.tile_pool(name="ps_tr", bufs=2,
                                              space="PSUM"))
        ps_y = es.enter_context(tc.tile_pool(name="ps_y", bufs=2,
                                             space="PSUM"))

        # --- small resident inputs ---
        epsb = const_pool.tile([128, 1], F32, tag="epsb")
        nc.vector.memset(epsb[:], LN_EPS)
        ident = const_pool.tile([128, 128], BF16, tag="ident")
        nc.scalar.dma_start(ident[:], ident_d[:])
        pwt_sb = const_pool.tile([128, 2, DIM], BF16, tag="pwt")
        nc.scalar.dma_start(pwt_sb[:], pwt_d[:])
        relsl_sb = const_pool.tile([128, N_TILES, DIM], BF16, tag="relsl")
        nc.scalar.dma_start(relsl_sb[:], relsl_d[:])


        # --- the fold stream, in PE consumption order (sync ring) ---
        xt_sb = const_pool.tile([128, totf, DIM], FP8, tag="xt")
        wt_sb = const_pool.tile([128, totf, TILE], FP8, tag="wt")
        out_sb = const_pool.tile([128, N_TILES, DIM], BF16, tag="out_sb")
        for t in list(range(1, N_TILES)) + [0]:
            ch = _chunks(t, f_ts[t])
            for i in range(len(ch) - 1):
                lo, hi = off[t] + ch[i], off[t] + ch[i + 1]
                nc.sync.dma_start(xt_sb[:, lo:hi, :],
                                  xt_d[:, lo * DIM:hi * DIM])
                nc.sync.dma_start(wt_sb[:, lo:hi, :],
                                  wt_d[:, lo * TILE:hi * TILE])

        def stats_phase(t, agg_ps):
            # one-pass mean/var from PSUM, then rstd
            stats = ep_pool.tile([128, 6], F32, tag="stats")
            nc.vector.bn_stats(stats[:], agg_ps[:])
            mv = ep_pool.tile([128, 2], F32, tag="mv")
            nc.vector.bn_aggr(mv[:], stats[:])
            std = ep_pool.tile([128, 1], F32, tag="std")
            nc.scalar.activation(std[:], mv[:, 1:2], AF.Sqrt, bias=epsb[:])
            rstd = ep_pool.tile([128, 1], F32, tag="rstd")
            nc.vector.reciprocal(rstd[:], std[:])
            return mv, rstd

        def apply_a(t, agg_ps, mv, rstd):
            # ln -> transpose -> y = ln @ proj_w.T (PSUM)
            ln = ep_pool.tile([128, DIM], BF16, tag="ln")
            nc.vector.tensor_scalar(ln[:], agg_ps[:], mv[:, 0:1], rstd[:],
                                    AluOpType.subtract, AluOpType.mult)
            y_ps = ps_y.tile([128, DIM], F32, tag="y")
            for k in range(2):
                tr_ps = ps_tr.tile([128, 128], BF16, tag="tr")
                nc.tensor.transpose(tr_ps[:], ln[:, k * 128:(k + 1) * 128],
                                    ident[:])
                lnT = ep_pool.tile([128, 128], BF16, tag="lnT")
                nc.scalar.copy(lnT[:], tr_ps[:])
                nc.tensor.matmul(y_ps[:], lnT[:], pwt_sb[:, k, :],
                                 start=(k == 0), stop=(k == 1))
            return y_ps

        def apply_b(t, y_ps):
            # fused alpha*y + rel, one iteration later so this op never
            # heads the DVE queue while waiting on the PE
            nc.vector.scalar_tensor_tensor(out_sb[:, t, :], y_ps[:], ALPHA,
                                           relsl_sb[:, t, :],
                                           AluOpType.mult, AluOpType.add)

        # three-deep software pipeline: every cross-engine dependency gets a
        # full tile-iteration of slack, so no engine queue ever stalls the
        # loop.  iter t runs: DRs(t) | apply(t-3) | stats(t-1)
        live = {}
        ys = {}
        for t in range(N_TILES):
            agg_ps = ps_agg.tile([128, DIM], F32, tag="agg")
            npair = f_ts[t] // 2
            for p in range(npair):
                lo = off[t] + 2 * p
                nc.tensor.matmul(agg_ps[:],
                                 wt_sb[:, lo:lo + 2, :],
                                 xt_sb[:, lo:lo + 2, :],
                                 start=(p == 0), stop=(p == npair - 1),
                                 perf_mode=mybir.MatmulPerfMode.DoubleRow)
            live[t] = [agg_ps]
            if t >= 1:
                live[t - 1] += stats_phase(t - 1, live[t - 1][0])
            if t >= 3:
                ys[t - 3] = apply_a(t - 3, *live.pop(t - 3))
            if t >= 4:
                apply_b(t - 4, ys.pop(t - 4))
        live[N_TILES - 1] += stats_phase(N_TILES - 1,
                                         live[N_TILES - 1][0])
        for t in range(N_TILES - 3, N_TILES):
            ys[t] = apply_a(t, *live.pop(t))
            apply_b(t - 1, ys.pop(t - 1))
            if t == N_TILES - 2:
                # tiles 0..7 are done: one fat store on the scalar ring
                nc.scalar.dma_start(out_d[:, :(N_TILES - 2) * DIM],
                                    out_sb[:, :N_TILES - 2, :])
        apply_b(N_TILES - 1, ys.pop(N_TILES - 1))
        nc.scalar.dma_start(out_d[:, (N_TILES - 2) * DIM:],
                            out_sb[:, N_TILES - 2:, :])

    nc.compile()
    return nc


def _prep(rel_embed, rel_edge_index, rel_edge_weight, proj_w):
    """Host-side sharding/layout: per (core, dst tile), compact the distinct
    src rows into fold tiles and interleave gathered X~ rows with their
    W~ weight rows in one stream."""
    src = np.asarray(rel_edge_index[0], dtype=np.int64)
    dst = np.asarray(rel_edge_index[1], dtype=np.int64)
    w = np.asarray(rel_edge_weight, dtype=np.float32)
    rel = np.asarray(rel_embed, dtype=np.float32)
    pw = np.asarray(proj_w, dtype=np.float32)

    rel8 = rel.astype(ml_dtypes.float8_e4m3)
    core = dst // NODES_PER_CORE
    drel = dst - core * NODES_PER_CORE
    tt = drel // TILE
    dd = drel % TILE

    order = np.lexsort((src, tt, core))
    so, to, co, do_, wo = (src[order], tt[order], core[order], dd[order],
                           w[order])
    grp = co * N_TILES + to
    starts = np.searchsorted(grp, np.arange(N_CORES * N_TILES))
    ends = np.append(starts[1:], len(grp))

    uniqs = {}
    f_ts = []
    for t in range(N_TILES):
        fmax = 0
        for c in range(N_CORES):
            a, b = starts[c * N_TILES + t], ends[c * N_TILES + t]
            u = np.unique(so[a:b])
            uniqs[c, t] = u
            fmax = max(fmax, -(-len(u) // TILE))
        f_ts.append(-(-fmax // 2) * 2)  # even fold count for DR pairs
    f_ts = tuple(f_ts)
    off = np.concatenate([[0], np.cumsum(f_ts)])
    totf = int(off[-1])

    xt_dev = np.zeros((N_CORES, totf * TILE, DIM), dtype=ml_dtypes.float8_e4m3)
    wt_dev = np.zeros((N_CORES, totf * TILE, TILE),
                      dtype=ml_dtypes.float8_e4m3)
    for c in range(N_CORES):
        for t in range(N_TILES):
            a, b = starts[c * N_TILES + t], ends[c * N_TILES + t]
            uniq = uniqs[c, t]
            f = f_ts[t]
            slot = np.searchsorted(uniq, so[a:b])
            wblk = np.bincount(slot * TILE + do_[a:b], weights=wo[a:b],
                               minlength=f * TILE * TILE)
            r0 = int(off[t]) * TILE
            wt_dev[c, r0:r0 + f * TILE] = wblk.reshape(
                f * TILE, TILE).astype(np.float32)
            xt_dev[c, r0:r0 + len(uniq)] = rel8[uniq]
    # fold-major -> [partition(i), fold*d]: row i of fold f is src slot
    # f*128+i
    xt_dev = np.ascontiguousarray(
        xt_dev.reshape(N_CORES, totf, TILE, DIM)
        .transpose(0, 2, 1, 3).reshape(N_CORES, 128, totf * DIM))
    wt_dev = np.ascontiguousarray(
        wt_dev.reshape(N_CORES, totf, TILE, TILE)
        .transpose(0, 2, 1, 3).reshape(N_CORES, 128, totf * TILE))

    rel16 = rel.astype(ml_dtypes.bfloat16)
    relsl = np.zeros((N_CORES, 128, N_TILES * DIM), dtype=ml_dtypes.bfloat16)
    for c in range(N_CORES):
        sl = np.zeros((OUT_ROWS, DIM), dtype=ml_dtypes.bfloat16)
        sl[:NODES_PER_CORE] = rel16[c * NODES_PER_CORE:
                                    (c + 1) * NODES_PER_CORE]
        relsl[c] = sl.reshape(N_TILES, 128, DIM).transpose(1, 0, 2).reshape(
            128, N_TILES * DIM)

    pwt = pw.T.astype(ml_dtypes.bfloat16)  # [f, o]
    pwt_dev = np.ascontiguousarray(
        pwt.reshape(2, 128, DIM).transpose(1, 0, 2).reshape(128, 2 * DIM))
    ident_dev = np.eye(128, dtype=ml_dtypes.bfloat16)

    in_maps = []
    for c in range(N_CORES):
        in_maps.append({
            "xt": xt_dev[c],
            "wt": wt_dev[c],
            "ident": ident_dev,
            "relslice": np.ascontiguousarray(relsl[c]),
            "projwT": pwt_dev,
        })
    return in_maps, f_ts


def kernel(rel_embed, rel_edge_index, rel_edge_weight, proj_w,
           _trace=False):
    in_maps, f_ts = _prep(rel_embed, rel_edge_index, rel_edge_weight,
                          proj_w)
    nc = _CACHE.get(f_ts)
    if nc is None:
        nc = _build(f_ts)
        _CACHE[f_ts] = nc
    res = run_bass_kernel_spmd(nc, in_maps, core_ids=list(range(N_CORES)),
                               trace=_trace)
    parts = []
    for c in range(N_CORES):
        o = np.asarray(res.results[c]["out"]).reshape(128, N_TILES, DIM)
        o = o.transpose(1, 0, 2).reshape(OUT_ROWS, DIM)[:NODES_PER_CORE]
        parts.append(o)
    out = np.concatenate(parts, axis=0)
    if _trace:
        kernel.last_results = res
    return out.astype(np.float32)


# revision 22
# speedup vs baseline: 1.0807x; 1.0807x over previous
"""DualRelGCN message-passing kernel for 8 TRN2 NeuronCores.

Strategy (destination-sharded, collective-free, distinct-src folding):
  - LayerNorm is invariant to positive per-row scaling, so LN(agg/denom) ==
    LN(agg): the denominator drops out of the computation entirely.
  - Shard edges by dst range: core c owns nodes [1250c, 1250(c+1)) and
    computes its 1250 output rows completely locally -> no collectives.
  - Per dst tile (128 dsts) only ~3.1k of the 10k src nodes have an edge in,
    so the host compacts the active src rows into <=28 "fold" tiles of 128:
    X~[fold] = gathered rel_embed rows (fp8), W~[fold] = [128 src x 128 dst]
    dense weight block over the compacted rows.  agg_t = sum_f W~_f.T X~_f.
    vs. the all-src dense formulation this is ~3x fewer PE block-matmuls
    (the PE runs DoubleRow fp8 at ~256 cycles per 2-tile pair on this hw)
    and ~15% fewer HBM bytes.
  - X~ row i and its weight row ride adjacent in one interleaved stream
    ([128, fold, 256+128] fp8), DMA'd in exact PE consumption order, fully
    SBUF-resident -> single full-bandwidth burst, no recycle stalls.
  - Epilogue per dst tile: LN on ACT/DVE, PE transpose, y = ln @ proj_w.T,
    out = rel + 0.1*y in bf16 (host upcasts to fp32).
"""

import sys

for _p in ("/opt/trn_rl_repo",):
    if _p not in sys.path:
        sys.path.insert(0, _p)

from contextlib import ExitStack

import numpy as np
import ml_dtypes

import concourse.bacc as bacc
import concourse.mybir as mybir
from concourse.alu_op_type import AluOpType
from concourse.tile import TileContext
from concourse.bass_utils import run_bass_kernel_spmd

F32 = mybir.dt.float32
BF16 = mybir.dt.bfloat16
FP8 = mybir.dt.float8e4
AF = mybir.ActivationFunctionType

N_NODES = 10000
DIM = 256
N_CORES = 8
NODES_PER_CORE = N_NODES // N_CORES  # 1250
TILE = 128
N_TILES = -(-NODES_PER_CORE // TILE)  # 10 dst tiles per core
OUT_ROWS = N_TILES * TILE  # 1280
ALPHA = 0.1
LN_EPS = 1e-5

_CACHE: dict = {}


def _chunks(t, f):
    # fold-unit chunk boundaries within tile t: halves, finer at the ends
    if t == 0:
        return [0, 8, f]
    if t == N_TILES - 1:
        return [0, f // 2, (f // 2 + f) // 2, f]
    return [0, f // 2, f]


def _build(f_ts):
    off = [0]
    for f in f_ts:
        off.append(off[-1] + f)
    totf = off[-1]
    nc = bacc.Bacc("TRN2", target_bir_lowering=False, debug=False,
                   num_devices=N_CORES)

    xt_d = nc.dram_tensor("xt", [128, totf * DIM], FP8,
                          kind="ExternalInput")
    wt_d = nc.dram_tensor("wt", [128, totf * TILE], FP8,
                          kind="ExternalInput")
    ident_d = nc.dram_tensor("ident", [128, 128], BF16, kind="ExternalInput")
    relsl_d = nc.dram_tensor("relslice", [128, N_TILES * DIM], BF16,
                             kind="ExternalInput")
    pwt_d = nc.dram_tensor("projwT", [128, 2 * DIM], BF16,
                           kind="ExternalInput")
    out_d = nc.dram_tensor("out", [128, N_TILES * DIM], BF16,
                           kind="ExternalOutput")

    with TileContext(nc) as tc, ExitStack() as es:
        const_pool = es.enter_context(tc.tile_pool(name="const", bufs=1))
        ep_pool = es.enter_context(tc.tile_pool(name="ep", bufs=3))
        ps_agg = es.enter_context(tc.tile_pool(name="ps_agg", bufs=4,
                                               space="PSUM"))
        ps_tr = es.enter_context(tc.tile_pool(name="ps_tr", bufs=2,
                                              space="PSUM"))
        ps_y = es.enter_context(tc.tile_pool(name="ps_y", bufs=2,
                                             space="PSUM"))

        # --- small resident inputs ---
        epsb = const_pool.tile([128, 1], F32, tag="epsb")
        nc.vector.memset(epsb[:], LN_EPS)
        ident = const_pool.tile([128, 128], BF16, tag="ident")
        nc.scalar.dma_start(ident[:], ident_d[:])
        pwt_sb = const_pool.tile([128, 2, DIM], BF16, tag="pwt")
        nc.scalar.dma_start(pwt_sb[:], pwt_d[:])
        relsl_sb = const_pool.tile([128, N_TILES, DIM], BF16, tag="relsl")
        nc.scalar.dma_start(relsl_sb[:], relsl_d[:])


        # --- the fold stream, in PE consumption order (sync ring) ---
        xt_sb = const_pool.tile([128, totf, DIM], FP8, tag="xt")
        wt_sb = const_pool.tile([128, totf, TILE], FP8, tag="wt")
        out_sb = const_pool.tile([128, N_TILES, DIM], BF16, tag="out_sb")
        for t in range(N_TILES):
            ch = _chunks(t, f_ts[t])
            for i in range(len(ch) - 1):
                lo, hi = off[t] + ch[i], off[t] + ch[i + 1]
                nc.sync.dma_start(xt_sb[:, lo:hi, :],
                                  xt_d[:, lo * DIM:hi * DIM])
                nc.sync.dma_start(wt_sb[:, lo:hi, :],
                                  wt_d[:, lo * TILE:hi * TILE])

        def stats_phase(t, agg_ps):
            # rowsum/sqsum via ACT accumulators (reads PSUM, spills agg to
            # SBUF for the later ln), var = E[x^2]-mu^2 on tiny DVE ops
            agg_sb = ep_pool.tile([128, DIM], F32, tag="aggsb")
            rowsum = ep_pool.tile([128, 1], F32, tag="rowsum")
            nc.scalar.activation(agg_sb[:], agg_ps[:], AF.Copy,
                                 accum_out=rowsum[:])
            sqscr = ep_pool.tile([128, DIM], F32, tag="sqscr")
            sqsum = ep_pool.tile([128, 1], F32, tag="sqsum")
            nc.scalar.activation(sqscr[:], agg_ps[:], AF.Square,
                                 accum_out=sqsum[:])
            mu = ep_pool.tile([128, 1], F32, tag="mu")
            nc.vector.tensor_scalar(mu[:], rowsum[:], 1.0 / DIM, None,
                                    AluOpType.mult)
            musq = ep_pool.tile([128, 1], F32, tag="musq")
            nc.vector.tensor_tensor(musq[:], mu[:], mu[:], AluOpType.mult)
            var = ep_pool.tile([128, 1], F32, tag="var")
            nc.vector.scalar_tensor_tensor(var[:], sqsum[:], 1.0 / DIM,
                                           musq[:], AluOpType.mult,
                                           AluOpType.subtract)
            std = ep_pool.tile([128, 1], F32, tag="std")
            nc.scalar.activation(std[:], var[:], AF.Sqrt, bias=epsb[:])
            rstd = ep_pool.tile([128, 1], F32, tag="rstd")
            nc.vector.reciprocal(rstd[:], std[:])
            return agg_sb, mu, rstd

        def apply_a(t, agg_sb, mu, rstd):
            # ln -> transpose -> y = ln @ proj_w.T (PSUM)
            ln = ep_pool.tile([128, DIM], BF16, tag="ln")
            nc.vector.tensor_scalar(ln[:], agg_sb[:], mu[:], rstd[:],
                                    AluOpType.subtract, AluOpType.mult)
            y_ps = ps_y.tile([128, DIM], F32, tag="y")
            for k in range(2):
                tr_ps = ps_tr.tile([128, 128], BF16, tag="tr")
                nc.tensor.transpose(tr_ps[:], ln[:, k * 128:(k + 1) * 128],
                                    ident[:])
                lnT = ep_pool.tile([128, 128], BF16, tag="lnT")
                nc.vector.tensor_scalar(lnT[:], tr_ps[:], 0.0, None,
                                        AluOpType.add)
                nc.tensor.matmul(y_ps[:], lnT[:], pwt_sb[:, k, :],
                                 start=(k == 0), stop=(k == 1))
            return y_ps

        def apply_b(t, y_ps):
            # fused alpha*y + rel, one iteration later so this op never
            # heads the DVE queue while waiting on the PE
            nc.vector.scalar_tensor_tensor(out_sb[:, t, :], y_ps[:], ALPHA,
                                           relsl_sb[:, t, :],
                                           AluOpType.mult, AluOpType.add)

        # three-deep software pipeline: every cross-engine dependency gets a
        # full tile-iteration of slack, so no engine queue ever stalls the
        # loop.  iter t runs: DRs(t) | apply(t-3) | stats(t-1)
        live = {}
        ys = {}
        for t in range(N_TILES):
            agg_ps = ps_agg.tile([128, DIM], F32, tag="agg")
            npair = f_ts[t] // 2
            for p in range(npair):
                lo = off[t] + 2 * p
                nc.tensor.matmul(agg_ps[:],
                                 wt_sb[:, lo:lo + 2, :],
                                 xt_sb[:, lo:lo + 2, :],
                                 start=(p == 0), stop=(p == npair - 1),
                                 perf_mode=mybir.MatmulPerfMode.DoubleRow)
            live[t] = [agg_ps]
            if t >= 1:
                live[t - 1] = list(stats_phase(t - 1, live[t - 1][0]))
            if t >= 3:
                ys[t - 3] = apply_a(t - 3, *live.pop(t - 3))
            if t >= 4:
                apply_b(t - 4, ys.pop(t - 4))
        live[N_TILES - 1] = list(stats_phase(N_TILES - 1,
                                             live[N_TILES - 1][0]))
        for t in range(N_TILES - 3, N_TILES):
            ys[t] = apply_a(t, *live.pop(t))
            apply_b(t - 1, ys.pop(t - 1))
            if t == N_TILES - 2:
                # tiles 0..7 are done: one fat store on the scalar ring
                nc.scalar.dma_start(out_d[:, :(N_TILES - 2) * DIM],
                                    out_sb[:, :N_TILES - 2, :])
        apply_b(N_TILES - 1, ys.pop(N_TILES - 1))
        nc.scalar.dma_start(out_d[:, (N_TILES - 2) * DIM:],
                            out_sb[:, N_TILES - 2:, :])

    nc.compile()
    return nc


def _prep(rel_embed, rel_edge_index, rel_edge_weight, proj_w):
    """Host-side sharding/layout: per (core, dst tile), compact the distinct
    src rows into fold tiles and interleave gathered X~ rows with their
    W~ weight rows in one stream."""
    src = np.asarray(rel_edge_index[0], dtype=np.int64)
    dst = np.asarray(rel_edge_index[1], dtype=np.int64)
    w = np.asarray(rel_edge_weight, dtype=np.float32)
    rel = np.asarray(rel_embed, dtype=np.float32)
    pw = np.asarray(proj_w, dtype=np.float32)

    rel8 = rel.astype(ml_dtypes.float8_e4m3)
    core = dst // NODES_PER_CORE
    drel = dst - core * NODES_PER_CORE
    tt = drel // TILE
    dd = drel % TILE

    order = np.lexsort((src, tt, core))
    so, to, co, do_, wo = (src[order], tt[order], core[order], dd[order],
                           w[order])
    grp = co * N_TILES + to
    starts = np.searchsorted(grp, np.arange(N_CORES * N_TILES))
    ends = np.append(starts[1:], len(grp))

    uniqs = {}
    f_ts = []
    for t in range(N_TILES):
        fmax = 0
        for c in range(N_CORES):
            a, b = starts[c * N_TILES + t], ends[c * N_TILES + t]
            u = np.unique(so[a:b])
            uniqs[c, t] = u
            fmax = max(fmax, -(-len(u) // TILE))
        f_ts.append(-(-fmax // 2) * 2)  # even fold count for DR pairs
    f_ts = tuple(f_ts)
    off = np.concatenate([[0], np.cumsum(f_ts)])
    totf = int(off[-1])

    xt_dev = np.zeros((N_CORES, totf * TILE, DIM), dtype=ml_dtypes.float8_e4m3)
    wt_dev = np.zeros((N_CORES, totf * TILE, TILE),
                      dtype=ml_dtypes.float8_e4m3)
    for c in range(N_CORES):
        for t in range(N_TILES):
            a, b = starts[c * N_TILES + t], ends[c * N_TILES + t]
            uniq = uniqs[c, t]
            f = f_ts[t]
            slot = np.searchsorted(uniq, so[a:b])
            wblk = np.bincount(slot * TILE + do_[a:b], weights=wo[a:b],
                               minlength=f * TILE * TILE)
            r0 = int(off[t]) * TILE
            wt_dev[c, r0:r0 + f * TILE] = wblk.reshape(
                f * TILE, TILE).astype(np.float32)
            xt_dev[c, r0:r0 + len(uniq)] = rel8[uniq]
    # fold-major -> [partition(i), fold*d]: row i of fold f is src slot
    # f*128+i
    xt_dev = np.ascontiguousarray(
        xt_dev.reshape(N_CORES, totf, TILE, DIM)
        .transpose(0, 2, 1, 3).reshape(N_CORES, 128, totf * DIM))
    wt_dev = np.ascontiguousarray(
        wt_dev.reshape(N_CORES, totf, TILE, TILE)
        .transpose(0, 2, 1, 3).reshape(N_CORES, 128, totf * TILE))

    rel16 = rel.astype(ml_dtypes.bfloat16)
    relsl = np.zeros((N_CORES, 128, N_TILES * DIM), dtype=ml_dtypes.bfloat16)
    for c in range(N_CORES):
        sl = np.zeros((OUT_ROWS, DIM), dtype=ml_dtypes.bfloat16)
        sl[:NODES_PER_CORE] = rel16[c * NODES_PER_CORE:
                                    (c + 1) * NODES_PER_CORE]
        relsl[c] = sl.reshape(N_TILES, 128, DIM).transpose(1, 0, 2).reshape(
            128, N_TILES * DIM)

    pwt = pw.T.astype(ml_dtypes.bfloat16)  # [f, o]
    pwt_dev = np.ascontiguousarray(
        pwt.reshape(2, 128, DIM).transpose(1, 0, 2).reshape(128, 2 * DIM))
    ident_dev = np.eye(128, dtype=ml_dtypes.bfloat16)

    in_maps = []
    for c in range(N_CORES):
        in_maps.append({
            "xt": xt_dev[c],
            "wt": wt_dev[c],
            "ident": ident_dev,
            "relslice": np.ascontiguousarray(relsl[c]),
            "projwT": pwt_dev,
        })
    return in_maps, f_ts


def kernel(rel_embed, rel_edge_index, rel_edge_weight, proj_w,
           _trace=False):
    in_maps, f_ts = _prep(rel_embed, rel_edge_index, rel_edge_weight,
                          proj_w)
    nc = _CACHE.get(f_ts)
    if nc is None:
        nc = _build(f_ts)
        _CACHE[f_ts] = nc
    res = run_bass_kernel_spmd(nc, in_maps, core_ids=list(range(N_CORES)),
                               trace=_trace)
    parts = []
    for c in range(N_CORES):
        o = np.asarray(res.results[c]["out"]).reshape(128, N_TILES, DIM)
        o = o.transpose(1, 0, 2).reshape(OUT_ROWS, DIM)[:NODES_PER_CORE]
        parts.append(o)
    out = np.concatenate(parts, axis=0)
    if _trace:
        kernel.last_results = res
    return out.astype(np.float32)
